# revision 22
# baseline (speedup 1.0000x reference)
"""CBAM channel attention kernel for Trainium2 (8 NeuronCores, batch-parallel).

x: [32, 768, 56, 56] f32 on host, cast to bf16 for the device pass (38.5 MB
HBM traffic per core round-trip, ~93 us fabric floor at 435 GB/s).  Each core
handles 4 samples; channel-chunk pairs [128, 2, 3136] stay resident in SBUF
between pooling and scaling so HBM traffic is exactly 1 read + 1 write of x.

The schedule is built around ScalarE (ACT), the throughput pacer: 24 sum-
pools (2.9us each) + 4 gate chains ~= 74us of ACT work.  Everything else is
phased (tile_wait_until as a priority key) to keep ACT dense:
 1. Sample 0's reads are per-chunk so the first sum-pool starts ~8us, and
    three of window 0's sum-pools run on DVE (add-tree + CACHE_REDUCE add)
    where DVE has fill-phase slack, cutting the window-0 ACT backlog.
 2. Sum-pools for chunks 0-3 of sample b+1 are phased into sample b's gate
    window as ACT bubble-fillers; ERF sits after the 3rd filler so matmul1's
    CACHE_REDUCE deps (pair-2 trees run FIRST on DVE) are ready, and the
    serial 12-op matmul2 chain hides under the 4th filler before SIGMOID.
 3. DVE interleaves sample b+1's max-tree pairs with sample b's gate
    multiplies (pair2-first) so writes flow while CRs stay ahead of ACT.
 4. The last sample runs a per-chunk matmul2 -> sigmoid -> multiply -> write
    pipeline (one chunk handed to the idle ACT), writes spread across all
    three DMA queues so the final dge_drain hides under HWDGE transfers.
 5. The 1/HW mean scale rides the sum-pool Copy's free affine; a dummy
    sigmoid pins the sigmoid table-set (holds erf+copy) so only one
    ACT_TABLE_LOAD happens.

Pooling: max on DVE as a depth-2 tensor_tensor max tree + one CACHE_REDUCE
per chunk; sum on ScalarE (Copy + accum_out, main output to a zero-stride
sink).  MLP in transposed form on TensorE with host-pretransposed f32
weights; exact gelu via Erf (0.5 folded into w2t).  Reads ride the Sync
HWDGE ring; steady-state writes ride SWDGE.
"""

import ml_dtypes
import numpy as np

import concourse.bacc as bacc
import concourse.bass as bass
import concourse.mybir as mybir
import concourse.tile as tile
from concourse.bass_utils import run_bass_kernel_spmd

B = 32
C = 768
HW = 56 * 56    # 3136
HWH = HW // 2   # 1568
HWQ = HW // 4   # 784
HID = 48        # C // 16
NCORES = 8
B_LOC = B // NCORES  # 4
KC = C // 128        # 6 channel chunks
F32 = mybir.dt.float32
BF16 = mybir.dt.bfloat16
AF = mybir.ActivationFunctionType
ALU = mybir.AluOpType

_cache = {}


def _build_nc():
    nc = bacc.Bacc("TRN2", target_bir_lowering=False, debug=False)
    x_d = nc.declare_dram_parameter("x", [B_LOC * C, HW], BF16, isOutput=False)
    # host-pretransposed weights: w1t[p, k, h] = w1[h, k*128+p],
    # w2t[h, k, p] = 0.5 * w2[k*128+p, h]  (0.5 folds the gelu half)
    w1_d = nc.declare_dram_parameter("w1t", [128, KC * HID], F32, isOutput=False)
    w2_d = nc.declare_dram_parameter("w2t", [HID, KC * 128], F32, isOutput=False)
    out_d = nc.declare_dram_parameter("out", [B_LOC * C, HW], BF16, isOutput=True)

    with tile.TileContext(nc) as tc:
        with (
            tc.tile_pool(name="consts", bufs=1) as consts,
            tc.tile_pool(name="big", bufs=10) as bigpool,
            tc.tile_pool(name="ttree", bufs=2) as tpool,
            tc.tile_pool(name="pooled", bufs=3) as pooled_pool,
            tc.tile_pool(name="small", bufs=3) as small_pool,
            tc.tile_pool(name="psum", bufs=2, space="PSUM") as psum_pool,
        ):
            sink = consts.tile([128, 1], BF16)
            # write-only scratch for the CACHE_REDUCE output streams
            garbage = consts.tile([128, HWQ], BF16)
            # dummy sigmoid: pin the sigmoid table-set (contains erf + copy)
            # before the first sum-pool so only ONE ACT_TABLE_LOAD happens
            with tc.tile_wait_until(0.001):
                nc.scalar.activation(out=sink[:, 0:1], in_=sink[:, 0:1],
                                     func=AF.Sigmoid)

            all_ots = []
            all_pooled = []

            # ---- reads: greedy, Sync HWDGE ring.  Sample 0 goes per-chunk
            # so the first pool ops start as soon as ~0.8 MB lands. ----
            for b in range(B_LOC):
                ots = []
                for j in range(KC // 2):
                    ot = bigpool.tile([128, 2, HW], BF16, tag="o", bufs=10,
                                      name=f"ot{b}_{j}")
                    row = (b * KC + 2 * j) * 128
                    if b == 0:
                        for i in range(2):
                            nc.sync.dma_start(
                                out=ot[:, i, :],
                                in_=x_d[row + 128 * i : row + 128 * (i + 1), :],
                            )
                    else:
                        nc.sync.dma_start(
                            out=ot,
                            in_=x_d[row : row + 256, :].rearrange(
                                "(k p) f -> p k f", p=128
                            ),
                        )
                    ots.append(ot)
                all_ots.append(ots)
                all_pooled.append(
                    pooled_pool.tile([128, KC, 2], F32, name=f"pooled{b}")
                )
                if b == 0:
                    # weights ride the sync ring AFTER sample 0's chunks:
                    # they aren't needed until matmul1 (~25us in), and
                    # triggering them first would delay the first pool ops
                    w1T = consts.tile([128, KC, HID], F32)
                    nc.sync.dma_start(
                        out=w1T, in_=w1_d.rearrange("p (k h) -> p k h", k=KC)
                    )
                    w2T = consts.tile([HID, KC, 128], F32)
                    nc.sync.dma_start(
                        out=w2T, in_=w2_d.rearrange("h (k p) -> h k p", k=KC)
                    )

            def act_sum(b, k, ph):
                with tc.tile_wait_until(ph):
                    nc.scalar.activation(
                        out=sink[:, 0:1].to_broadcast([128, HW]),
                        in_=all_ots[b][k // 2][:, k % 2, :],
                        func=AF.Copy,
                        scale=1.0 / HW,
                        accum_out=all_pooled[b][:, k, 0:1],
                    )

            def gp_sum(b, k, gp_ph, act_ph):
                # first add-tree level on the otherwise-idle GpSimd, so the
                # ACT finish reads 1568 wide instead of 3136
                src = all_ots[b][k // 2][:, k % 2, :]
                g1 = tpool.tile([128, HWH], BF16, tag="g1", bufs=2,
                                name=f"g1_{b}_{k}")
                with tc.tile_wait_until(gp_ph):
                    nc.gpsimd.tensor_tensor(
                        out=g1, in0=src[0:128, 0:HWH], in1=src[0:128, HWH:HW],
                        op=ALU.add,
                    )
                with tc.tile_wait_until(act_ph):
                    nc.scalar.activation(
                        out=sink[:, 0:1].to_broadcast([128, HWH]),
                        in_=g1,
                        func=AF.Copy,
                        scale=1.0 / HW,
                        accum_out=all_pooled[b][:, k, 0:1],
                    )

            def dve_sum(b, k, ph):
                # add-tree + CACHE_REDUCE(add): sum-pool on DVE for the
                # fill phase where DVE has slack and ACT is the backlog.
                # bf16 intermediates round ~2^-9 per level; the CR
                # accumulates in f32, well inside the 2e-2 error budget.
                with tc.tile_wait_until(ph):
                    src = all_ots[b][k // 2][:, k % 2, :]
                    s1 = tpool.tile([128, HWH], BF16, tag="s1", bufs=1,
                                    name=f"s1_{b}_{k}")
                    nc.vector.tensor_tensor(
                        out=s1, in0=src[0:128, 0:HWH], in1=src[0:128, HWH:HW],
                        op=ALU.add,
                    )
                    s2 = tpool.tile([128, HWQ], BF16, tag="s2", bufs=1,
                                    name=f"s2_{b}_{k}")
                    nc.vector.tensor_tensor(
                        out=s2, in0=s1[:, 0:HWQ], in1=s1[:, HWQ:HWH],
                        op=ALU.add,
                    )
                    nc.vector.tensor_scalar(
                        out=garbage[:, 0:HWQ],
                        in0=s2,
                        scalar1=1.0 / HW,
                        scalar2=None,
                        op0=ALU.mult,
                        op1=ALU.add,
                        accum_out=all_pooled[b][:, k, 0:1],
                    )

            HWE = HWQ // 2  # 392

            def max_tree(b, j, ph):
                with tc.tile_wait_until(ph):
                    ot = all_ots[b][j]
                    t1 = tpool.tile([128, 2, HWH], BF16, tag="t1", bufs=1,
                                    name=f"t1_{b}_{j}")
                    nc.vector.tensor_tensor(
                        out=t1, in0=ot[:, :, 0:HWH], in1=ot[:, :, HWH:HW],
                        op=ALU.max,
                    )
                    t2 = tpool.tile([128, 2, HWQ], BF16, tag="t2", bufs=1,
                                    name=f"t2_{b}_{j}")
                    nc.vector.tensor_tensor(
                        out=t2, in0=t1[:, :, 0:HWQ], in1=t1[:, :, HWQ:HWH],
                        op=ALU.max,
                    )
                    t3 = tpool.tile([128, 2, HWE], BF16, tag="t3", bufs=1,
                                    name=f"t3_{b}_{j}")
                    nc.vector.tensor_tensor(
                        out=t3, in0=t2[:, :, 0:HWE], in1=t2[:, :, HWE:HWQ],
                        op=ALU.max,
                    )
                    # one segmented reduce finishes BOTH chunks of the pair
                    nc.vector.tensor_reduce(
                        out=all_pooled[b][:, 2 * j : 2 * j + 2, 1],
                        in_=t3,
                        axis=mybir.AxisListType.X,
                        op=ALU.max,
                    )

            # ---- per-sample emission ----
            for b in range(B_LOC):
                ots = all_ots[b]
                pooled = all_pooled[b]
                last = b == B_LOC - 1

                # ACT sum-pools.  Window 0: chunks 0-4 on ACT in read-arrival
                # order, chunk 5 on DVE after the trees.  Steady state:
                # chunks 0-3 of sample b are fillers in window b-1's gate
                # chain; chunks 4,5 stay in window b.
                if b == 0:
                    for k in range(5):
                        act_sum(0, k, 0.01 + 0.01 * k)
                else:
                    for k in range(2):
                        gp_sum(b, k, (b - 1) + 0.86 + 0.01 * k,
                               (b - 1) + (0.90, 0.92)[k])
                    for k in range(2, 4):
                        act_sum(b, k, (b - 1) + (0.94, 0.95)[k - 2])
                    for k in range(4, KC):
                        act_sum(b, k, b + 0.10 + 0.02 * (k - 4))

                # DVE max-trees in read-arrival order, interleaved with the
                # PREVIOUS sample's gate multiplies
                if b == 0:
                    for j in range(KC // 2):
                        max_tree(0, j, 0.06 + 0.005 * j)
                    dve_sum(0, 5, 0.075)
                else:
                    for j in range(KC // 2):
                        max_tree(b, j, b + 0.10 + 0.02 * j)

                # matmul1: hT [48, 2] = sum_k w1T_k.T @ pooledT_k
                hps = psum_pool.tile([HID, 2], F32, tag="hps", name=f"hps{b}")
                for k in range(KC):
                    with tc.tile_wait_until(b + 0.30 + 0.01 * k):
                        nc.tensor.matmul(
                            hps,
                            w1T[:, k, :],
                            pooled[:, k, :],
                            start=(k == 0),
                            stop=(k == KC - 1),
                        )

                # gate chain: erf -> hh/hsum (DVE stt) -> matmul2 -> sigmoid.
                # Window 0's deps are ready early, so its erf/sigmoid slot
                # earlier among the fillers to pull sample 0's writes in.
                erf_ph = 0.91 if b == 0 else b + 0.945
                with tc.tile_wait_until(erf_ph):
                    e_sb = small_pool.tile([HID, 2], F32, tag="e",
                                           name=f"e{b}")
                    nc.scalar.activation(
                        out=e_sb, in_=hps, func=AF.Erf, scale=0.7071067811865476
                    )
                with tc.tile_wait_until(erf_ph + 0.002):
                    # hh = (e + 1) * u; gate path is linear in hh, so accum_out
                    # sums avg+max columns directly into hsum for matmul2
                    hh = small_pool.tile([HID, 2], F32, tag="hh", name=f"hh{b}")
                    hsum = small_pool.tile([HID, 1], F32, tag="hsum",
                                           name=f"hsum{b}")
                    nc.vector.scalar_tensor_tensor(
                        out=hh, in0=e_sb, scalar=1.0, in1=hps,
                        op0=ALU.add, op1=ALU.mult, accum_out=hsum,
                    )
                mlp = psum_pool.tile([128, KC], F32, tag="mlp", name=f"mlp{b}")
                gate = small_pool.tile([128, KC], F32, tag="gate",
                                       name=f"gate{b}")
                sig_ph = 0.93 if b == 0 else b + 0.965
                if not last:
                    for k in range(KC):
                        with tc.tile_wait_until(erf_ph + 0.004 + 0.001 * k):
                            nc.tensor.matmul(
                                mlp[:, k : k + 1],
                                w2T[:, k, :],
                                hsum,
                                start=True,
                                stop=True,
                            )
                    with tc.tile_wait_until(sig_ph):
                        nc.scalar.activation(out=gate, in_=mlp, func=AF.Sigmoid)

                    # multiplies + writes: window b+1, interleaved after the
                    # corresponding tree pair of sample b+1 on DVE
                    for j in range(KC // 2):
                        with tc.tile_wait_until(b + 1.11 + 0.02 * j):
                            ot = ots[j]
                            row = (b * KC + 2 * j) * 128
                            wt = bigpool.tile([128, 2, HW], BF16, tag="w",
                                              bufs=4, name=f"wt{b}_{j}")
                            for i in range(2):
                                k = 2 * j + i
                                nc.vector.tensor_scalar_mul(
                                    wt[:, i, :], ot[:, i, :], gate[:, k : k + 1]
                                )
                            out_ap = out_d[row : row + 256, :].rearrange(
                                "(k p) f -> p k f", p=128
                            )
                            nc.gpsimd.dma_start(out=out_ap, in_=wt)
                else:
                    # last sample: per-chunk matmul2 -> sigmoid -> mult ->
                    # write pipeline; chunk 4's mult goes to the idle ACT;
                    # writes spread across gpsimd/sync/scalar queues.  Write
                    # tiles reuse the steady-state "w" pair tag, half each.
                    wts = [
                        bigpool.tile([128, 2, HW], BF16, tag="w", bufs=4,
                                     name=f"wtl{j}")
                        for j in range(KC // 2)
                    ]
                    for k in range(KC):
                        with tc.tile_wait_until(b + 0.95 + 0.002 * k):
                            nc.tensor.matmul(
                                mlp[:, k : k + 1],
                                w2T[:, k, :],
                                hsum,
                                start=True,
                                stop=True,
                            )
                            nc.scalar.activation(
                                out=gate[:, k : k + 1], in_=mlp[:, k : k + 1],
                                func=AF.Sigmoid,
                            )
                        with tc.tile_wait_until(b + 0.96 + 0.002 * k):
                            ot = ots[k // 2]
                            row = (b * KC + k) * 128
                            wt = wts[k // 2][:, k % 2, :]
                            nc.vector.tensor_scalar_mul(
                                wt, ot[:, k % 2, :], gate[:, k : k + 1]
                            )
                            eng = (nc.gpsimd, nc.gpsimd, nc.sync, nc.sync,
                                   nc.scalar, nc.scalar)[k]
                            eng.dma_start(
                                out=out_d[row : row + 128, :], in_=wt
                            )
    nc.finalize()
    return nc


def kernel(x, w1, w2, _trace=False):
    if "nc" not in _cache:
        _cache["nc"] = _build_nc()
    nc = _cache["nc"]

    x = np.asarray(x).reshape(B, C, HW)
    w1t = np.ascontiguousarray(
        np.asarray(w1, np.float32).reshape(HID, KC, 128).transpose(2, 1, 0)
        .reshape(128, KC * HID)
    )
    w2t = np.ascontiguousarray(
        (0.5 * np.asarray(w2, np.float32)).reshape(KC, 128, HID)
        .transpose(2, 0, 1).reshape(HID, KC * 128)
    )
    in_maps = [
        {
            "x": np.ascontiguousarray(
                x[i * B_LOC : (i + 1) * B_LOC].reshape(B_LOC * C, HW)
            ).astype(ml_dtypes.bfloat16),
            "w1t": w1t,
            "w2t": w2t,
        }
        for i in range(NCORES)
    ]
    res = run_bass_kernel_spmd(nc, in_maps, core_ids=list(range(NCORES)),
                               trace=_trace)
    out = np.concatenate(
        [
            r["out"].astype(np.float32).reshape(B_LOC, C, 56, 56)
            for r in res.results
        ],
        axis=0,
    )
    if _trace:
        _cache["last_results"] = res
    return out


# revision 23
# speedup vs baseline: 1.2077x; 1.2077x over previous
"""CBAM channel attention kernel for Trainium2 (8 NeuronCores, batch-parallel).

x: [32, 768, 56, 56] f32 on host, cast to bf16 for the device pass (38.5 MB
HBM traffic per core round-trip, ~93 us fabric floor at 435 GB/s).  Each core
handles 4 samples; channel-chunk pairs [128, 2, 3136] stay resident in SBUF
between pooling and scaling so HBM traffic is exactly 1 read + 1 write of x.

The schedule is built around ScalarE (ACT), the throughput pacer: 24 sum-
pools (2.9us each) + 4 gate chains ~= 74us of ACT work.  Everything else is
phased (tile_wait_until as a priority key) to keep ACT dense:
 1. Sample 0's reads are per-chunk so the first sum-pool starts ~8us, and
    three of window 0's sum-pools run on DVE (add-tree + CACHE_REDUCE add)
    where DVE has fill-phase slack, cutting the window-0 ACT backlog.
 2. Sum-pools for chunks 0-3 of sample b+1 are phased into sample b's gate
    window as ACT bubble-fillers; ERF sits after the 3rd filler so matmul1's
    CACHE_REDUCE deps (pair-2 trees run FIRST on DVE) are ready, and the
    serial 12-op matmul2 chain hides under the 4th filler before SIGMOID.
 3. DVE interleaves sample b+1's max-tree pairs with sample b's gate
    multiplies (pair2-first) so writes flow while CRs stay ahead of ACT.
 4. The last sample runs a per-chunk matmul2 -> sigmoid -> multiply -> write
    pipeline (one chunk handed to the idle ACT), writes spread across all
    three DMA queues so the final dge_drain hides under HWDGE transfers.
 5. The 1/HW mean scale rides the sum-pool Copy's free affine; a dummy
    sigmoid pins the sigmoid table-set (holds erf+copy) so only one
    ACT_TABLE_LOAD happens.

Pooling: max on DVE as a depth-2 tensor_tensor max tree + one CACHE_REDUCE
per chunk; sum on ScalarE (Copy + accum_out, main output to a zero-stride
sink).  MLP in transposed form on TensorE with host-pretransposed f32
weights; exact gelu via Erf (0.5 folded into w2t).  Reads ride the Sync
HWDGE ring; steady-state writes ride SWDGE.
"""

import ml_dtypes
import numpy as np

import concourse.bacc as bacc
import concourse.bass as bass
import concourse.mybir as mybir
import concourse.tile as tile
from concourse.bass_utils import run_bass_kernel_spmd

B = 32
C = 768
HW = 56 * 56    # 3136
HWH = HW // 2   # 1568
HWQ = HW // 4   # 784
HID = 48        # C // 16
NCORES = 8
B_LOC = B // NCORES  # 4
KC = C // 128        # 6 channel chunks
F32 = mybir.dt.float32
BF16 = mybir.dt.bfloat16
AF = mybir.ActivationFunctionType
ALU = mybir.AluOpType

_cache = {}


def _build_nc():
    nc = bacc.Bacc("TRN2", target_bir_lowering=False, debug=False)
    x_d = nc.declare_dram_parameter("x", [B_LOC * C, HW], BF16, isOutput=False)
    # host-pretransposed weights: w1t[p, k, h] = w1[h, k*128+p],
    # w2t[h, k, p] = 0.5 * w2[k*128+p, h]  (0.5 folds the gelu half)
    w1_d = nc.declare_dram_parameter("w1t", [128, KC * HID], F32, isOutput=False)
    w2_d = nc.declare_dram_parameter("w2t", [HID, KC * 128], F32, isOutput=False)
    out_d = nc.declare_dram_parameter("out", [B_LOC * C, HW], BF16, isOutput=True)

    with tile.TileContext(nc) as tc:
        with (
            tc.tile_pool(name="consts", bufs=1) as consts,
            tc.tile_pool(name="big", bufs=10) as bigpool,
            tc.tile_pool(name="ttree", bufs=2) as tpool,
            tc.tile_pool(name="pooled", bufs=3) as pooled_pool,
            tc.tile_pool(name="small", bufs=3) as small_pool,
            tc.tile_pool(name="psum", bufs=2, space="PSUM") as psum_pool,
        ):
            sink = consts.tile([128, 1], BF16)
            # write-only scratch for the CACHE_REDUCE output streams
            garbage = consts.tile([128, HWQ], BF16)
            # dummy sigmoid: pin the sigmoid table-set (contains erf + copy)
            # before the first sum-pool so only ONE ACT_TABLE_LOAD happens
            with tc.tile_wait_until(0.001):
                nc.scalar.activation(out=sink[:, 0:1], in_=sink[:, 0:1],
                                     func=AF.Sigmoid)

            all_ots = []
            all_pooled = []

            # ---- reads: greedy, Sync HWDGE ring.  Sample 0 goes per-chunk
            # so the first pool ops start as soon as ~0.8 MB lands. ----
            for b in range(B_LOC):
                ots = []
                for j in range(KC // 2):
                    ot = bigpool.tile([128, 2, HW], BF16, tag="o", bufs=10,
                                      name=f"ot{b}_{j}")
                    row = (b * KC + 2 * j) * 128
                    if b == 0:
                        for i in range(2):
                            nc.sync.dma_start(
                                out=ot[:, i, :],
                                in_=x_d[row + 128 * i : row + 128 * (i + 1), :],
                            )
                    else:
                        nc.sync.dma_start(
                            out=ot,
                            in_=x_d[row : row + 256, :].rearrange(
                                "(k p) f -> p k f", p=128
                            ),
                        )
                    ots.append(ot)
                all_ots.append(ots)
                all_pooled.append(
                    pooled_pool.tile([128, KC, 2], F32, name=f"pooled{b}")
                )
                if b == 0:
                    # weights ride the sync ring AFTER sample 0's chunks:
                    # they aren't needed until matmul1 (~25us in), and
                    # triggering them first would delay the first pool ops
                    w1T = consts.tile([128, KC, HID], F32)
                    nc.sync.dma_start(
                        out=w1T, in_=w1_d.rearrange("p (k h) -> p k h", k=KC)
                    )
                    w2T = consts.tile([HID, KC, 128], F32)
                    nc.sync.dma_start(
                        out=w2T, in_=w2_d.rearrange("h (k p) -> h k p", k=KC)
                    )

            def act_sum(b, k, ph):
                with tc.tile_wait_until(ph):
                    nc.scalar.activation(
                        out=sink[:, 0:1].to_broadcast([128, HW]),
                        in_=all_ots[b][k // 2][:, k % 2, :],
                        func=AF.Copy,
                        scale=1.0 / HW,
                        accum_out=all_pooled[b][:, k, 0:1],
                    )

            def gp_sum(b, k, gp_ph, act_ph):
                # first add-tree level on the otherwise-idle GpSimd, so the
                # ACT finish reads 1568 wide instead of 3136
                src = all_ots[b][k // 2][:, k % 2, :]
                g1 = tpool.tile([128, HWH], BF16, tag="g1", bufs=2,
                                name=f"g1_{b}_{k}")
                with tc.tile_wait_until(gp_ph):
                    nc.gpsimd.tensor_tensor(
                        out=g1, in0=src[0:128, 0:HWH], in1=src[0:128, HWH:HW],
                        op=ALU.add,
                    )
                with tc.tile_wait_until(act_ph):
                    nc.scalar.activation(
                        out=sink[:, 0:1].to_broadcast([128, HWH]),
                        in_=g1,
                        func=AF.Copy,
                        scale=1.0 / HW,
                        accum_out=all_pooled[b][:, k, 0:1],
                    )

            def dve_sum(b, k, ph):
                # add-tree + CACHE_REDUCE(add): sum-pool on DVE for the
                # fill phase where DVE has slack and ACT is the backlog.
                # bf16 intermediates round ~2^-9 per level; the CR
                # accumulates in f32, well inside the 2e-2 error budget.
                with tc.tile_wait_until(ph):
                    src = all_ots[b][k // 2][:, k % 2, :]
                    s1 = tpool.tile([128, HWH], BF16, tag="s1", bufs=1,
                                    name=f"s1_{b}_{k}")
                    nc.vector.tensor_tensor(
                        out=s1, in0=src[0:128, 0:HWH], in1=src[0:128, HWH:HW],
                        op=ALU.add,
                    )
                    s2 = tpool.tile([128, HWQ], BF16, tag="s2", bufs=1,
                                    name=f"s2_{b}_{k}")
                    nc.vector.tensor_tensor(
                        out=s2, in0=s1[:, 0:HWQ], in1=s1[:, HWQ:HWH],
                        op=ALU.add,
                    )
                    nc.vector.tensor_scalar(
                        out=garbage[:, 0:HWQ],
                        in0=s2,
                        scalar1=1.0 / HW,
                        scalar2=None,
                        op0=ALU.mult,
                        op1=ALU.add,
                        accum_out=all_pooled[b][:, k, 0:1],
                    )

            HWE = HWQ // 2  # 392

            def max_tree(b, j, ph):
                with tc.tile_wait_until(ph):
                    ot = all_ots[b][j]
                    t1 = tpool.tile([128, 2, HWH], BF16, tag="t1", bufs=1,
                                    name=f"t1_{b}_{j}")
                    nc.vector.tensor_tensor(
                        out=t1, in0=ot[:, :, 0:HWH], in1=ot[:, :, HWH:HW],
                        op=ALU.max,
                    )
                    t2 = tpool.tile([128, 2, HWQ], BF16, tag="t2", bufs=1,
                                    name=f"t2_{b}_{j}")
                    nc.vector.tensor_tensor(
                        out=t2, in0=t1[:, :, 0:HWQ], in1=t1[:, :, HWQ:HWH],
                        op=ALU.max,
                    )
                    t3 = tpool.tile([128, 2, HWE], BF16, tag="t3", bufs=1,
                                    name=f"t3_{b}_{j}")
                    nc.vector.tensor_tensor(
                        out=t3, in0=t2[:, :, 0:HWE], in1=t2[:, :, HWE:HWQ],
                        op=ALU.max,
                    )
                    # one segmented reduce finishes BOTH chunks of the pair
                    nc.vector.tensor_reduce(
                        out=all_pooled[b][:, 2 * j : 2 * j + 2, 1],
                        in_=t3,
                        axis=mybir.AxisListType.X,
                        op=ALU.max,
                    )

            # ---- per-sample emission ----
            for b in range(B_LOC):
                ots = all_ots[b]
                pooled = all_pooled[b]
                last = b == B_LOC - 1

                # ACT sum-pools.  Window 0: chunks 0-4 on ACT in read-arrival
                # order, chunk 5 on DVE after the trees.  Steady state:
                # chunks 0-3 of sample b are fillers in window b-1's gate
                # chain; chunks 4,5 stay in window b.
                if b == 0:
                    for k in range(5):
                        act_sum(0, k, 0.01 + 0.01 * k)
                else:
                    for k in range(4):
                        act_sum(b, k, (b - 1) + (0.90, 0.92, 0.94, 0.95)[k])
                    for k in range(4, KC):
                        act_sum(b, k, b + 0.10 + 0.02 * (k - 4))

                # DVE max-trees in read-arrival order, interleaved with the
                # PREVIOUS sample's gate multiplies
                if b == 0:
                    for j in range(KC // 2):
                        max_tree(0, j, 0.06 + 0.005 * j)
                    dve_sum(0, 5, 0.075)
                else:
                    for j in range(KC // 2):
                        max_tree(b, j, b + 0.10 + 0.02 * j)

                # matmul1: hT [48, 2] = sum_k w1T_k.T @ pooledT_k
                hps = psum_pool.tile([HID, 2], F32, tag="hps", name=f"hps{b}")
                for k in range(KC):
                    with tc.tile_wait_until(b + 0.30 + 0.01 * k):
                        nc.tensor.matmul(
                            hps,
                            w1T[:, k, :],
                            pooled[:, k, :],
                            start=(k == 0),
                            stop=(k == KC - 1),
                        )

                # gate chain: erf -> hh/hsum (DVE stt) -> matmul2 -> sigmoid.
                # Window 0's deps are ready early, so its erf/sigmoid slot
                # earlier among the fillers to pull sample 0's writes in.
                erf_ph = 0.91 if b == 0 else b + 0.945
                with tc.tile_wait_until(erf_ph):
                    e_sb = small_pool.tile([HID, 2], F32, tag="e",
                                           name=f"e{b}")
                    nc.scalar.activation(
                        out=e_sb, in_=hps, func=AF.Erf, scale=0.7071067811865476
                    )
                with tc.tile_wait_until(erf_ph + 0.002):
                    # hh = (e + 1) * u; gate path is linear in hh, so accum_out
                    # sums avg+max columns directly into hsum for matmul2
                    hh = small_pool.tile([HID, 2], F32, tag="hh", name=f"hh{b}")
                    hsum = small_pool.tile([HID, 1], F32, tag="hsum",
                                           name=f"hsum{b}")
                    nc.vector.scalar_tensor_tensor(
                        out=hh, in0=e_sb, scalar=1.0, in1=hps,
                        op0=ALU.add, op1=ALU.mult, accum_out=hsum,
                    )
                mlp = psum_pool.tile([128, KC], F32, tag="mlp", name=f"mlp{b}")
                gate = small_pool.tile([128, KC], F32, tag="gate",
                                       name=f"gate{b}")
                sig_ph = 0.93 if b == 0 else b + 0.965
                if not last:
                    for k in range(KC):
                        with tc.tile_wait_until(erf_ph + 0.004 + 0.001 * k):
                            nc.tensor.matmul(
                                mlp[:, k : k + 1],
                                w2T[:, k, :],
                                hsum,
                                start=True,
                                stop=True,
                            )
                    with tc.tile_wait_until(sig_ph):
                        nc.scalar.activation(out=gate, in_=mlp, func=AF.Sigmoid)

                    # multiplies + writes: window b+1, interleaved after the
                    # corresponding tree pair of sample b+1 on DVE
                    for j in range(KC // 2):
                        with tc.tile_wait_until(b + 1.11 + 0.02 * j):
                            ot = ots[j]
                            row = (b * KC + 2 * j) * 128
                            wt = bigpool.tile([128, 2, HW], BF16, tag="w",
                                              bufs=4, name=f"wt{b}_{j}")
                            for i in range(2):
                                k = 2 * j + i
                                nc.vector.tensor_scalar_mul(
                                    wt[:, i, :], ot[:, i, :], gate[:, k : k + 1]
                                )
                            out_ap = out_d[row : row + 256, :].rearrange(
                                "(k p) f -> p k f", p=128
                            )
                            nc.gpsimd.dma_start(out=out_ap, in_=wt)
                else:
                    # last sample: per-chunk matmul2 -> sigmoid -> mult ->
                    # write pipeline; chunk 4's mult goes to the idle ACT;
                    # writes spread across gpsimd/sync/scalar queues.  Write
                    # tiles reuse the steady-state "w" pair tag, half each.
                    wts = [
                        bigpool.tile([128, 2, HW], BF16, tag="w", bufs=4,
                                     name=f"wtl{j}")
                        for j in range(KC // 2)
                    ]
                    for k in range(KC):
                        with tc.tile_wait_until(b + 0.95 + 0.002 * k):
                            nc.tensor.matmul(
                                mlp[:, k : k + 1],
                                w2T[:, k, :],
                                hsum,
                                start=True,
                                stop=True,
                            )
                            nc.scalar.activation(
                                out=gate[:, k : k + 1], in_=mlp[:, k : k + 1],
                                func=AF.Sigmoid,
                            )
                        with tc.tile_wait_until(b + 0.96 + 0.002 * k):
                            ot = ots[k // 2]
                            row = (b * KC + k) * 128
                            wt = wts[k // 2][:, k % 2, :]
                            nc.vector.tensor_scalar_mul(
                                wt, ot[:, k % 2, :], gate[:, k : k + 1]
                            )
                            eng = (nc.gpsimd, nc.gpsimd, nc.sync, nc.sync,
                                   nc.scalar, nc.scalar)[k]
                            eng.dma_start(
                                out=out_d[row : row + 128, :], in_=wt
                            )
    nc.finalize()
    return nc


def kernel(x, w1, w2, _trace=False):
    if "nc" not in _cache:
        _cache["nc"] = _build_nc()
    nc = _cache["nc"]

    x = np.asarray(x).reshape(B, C, HW)
    w1t = np.ascontiguousarray(
        np.asarray(w1, np.float32).reshape(HID, KC, 128).transpose(2, 1, 0)
        .reshape(128, KC * HID)
    )
    w2t = np.ascontiguousarray(
        (0.5 * np.asarray(w2, np.float32)).reshape(KC, 128, HID)
        .transpose(2, 0, 1).reshape(HID, KC * 128)
    )
    in_maps = [
        {
            "x": np.ascontiguousarray(
                x[i * B_LOC : (i + 1) * B_LOC].reshape(B_LOC * C, HW)
            ).astype(ml_dtypes.bfloat16),
            "w1t": w1t,
            "w2t": w2t,
        }
        for i in range(NCORES)
    ]
    res = run_bass_kernel_spmd(nc, in_maps, core_ids=list(range(NCORES)),
                               trace=_trace)
    out = np.concatenate(
        [
            r["out"].astype(np.float32).reshape(B_LOC, C, 56, 56)
            for r in res.results
        ],
        axis=0,
    )
    if _trace:
        _cache["last_results"] = res
    return out
